# revision 18
# baseline (speedup 1.0000x reference)
"""MoE feed-forward block (shared expert + top-2-of-8 routed experts) on 8
Trainium2 NeuronCores.

Sharding: expert-parallel with host-side routing (the dispatch/gather is part
of the sharding step). The host computes the top-2 gate in fp32, gathers each
expert's selected tokens (capacity-padded to a common C), and core c runs
ONLY expert c's matmuls on its ~C gathered tokens — instead of the dense
all-token compute — plus a 1/8 hidden-dim slice of the shared expert over all
tokens. Host combine: sum the 8 shared partials, scatter-add the gate-scaled
routed outputs by token index (gate scaling on host keeps the device mm2 free
to emit a transposed [d, token] layout).

Matmuls run in bf16 (fp32 accumulation in PSUM). Per-core tensor work is
~13.4 GMAC; every weight/activation byte is streamed from HBM exactly once.

Device layout:
  shared mm1: ssh.T[HS,TC] = sw1T[D,HS].T @ x.T[D,TC]    (lhsT=sw1T resident)
  shared mm2: y[TC,D]      = ssh.T[HS,TC].T @ sw2T[HS,D] (lhsT=ssh.T)
  routed mm1: sh.T[H,C]    = w1T[D,H].T @ xg.T[D,C]      (w1 streamed once,
                                                          all chunks per tile)
  routed mm2: yT[D,C]      = w2T[H,D].T @ sh.T[H,C]      (lhsT=w2 stationary,
                                         reused across token chunks; output
                                         transposed so tokens are the free dim)
"""

import ml_dtypes
import numpy as np

import concourse.bass as bass
import concourse.mybir as mybir
import concourse.tile as tile
from concourse import bacc
from concourse.bass import ds, ts
from concourse.bass_utils import run_bass_kernel_spmd

BF16 = ml_dtypes.bfloat16

D_MODEL = 1024
HIDDEN = 4096
N_EXP = 8
N_CORES = 8
TOP_K = 2
T = 4096                      # 2 * 2048 tokens
HS = HIDDEN // N_CORES        # shared-expert hidden slice per core
TC = 512                      # token chunk
P = 128

LAST_EXEC_NS = None
LAST_RESULT = None


def _build_nc(C):
    fp32 = mybir.dt.float32
    bf16 = mybir.dt.bfloat16
    AF = mybir.ActivationFunctionType

    nc = bacc.Bacc()
    xbf = nc.declare_dram_parameter("xbf", [P, 8, T], bf16, isOutput=False)
    xg = nc.declare_dram_parameter("xg", [P, 8, C], bf16, isOutput=False)
    w1t = nc.declare_dram_parameter("w1t", [P, 8, HIDDEN], bf16, isOutput=False)
    # w2 reordered on host: [P, nh=8, k=32, 128] so each nh slice is one
    # contiguous 1MB DMA
    w2r = nc.declare_dram_parameter("w2r", [P, 8, 32 * P], bf16, isOutput=False)
    sw1t = nc.declare_dram_parameter("sw1t", [P, 8, HS], bf16, isOutput=False)
    sw2 = nc.declare_dram_parameter("sw2", [P, 4, D_MODEL], bf16, isOutput=False)
    out_sh = nc.declare_dram_parameter("out_sh", [T, D_MODEL], bf16, isOutput=True)
    out_rt = nc.declare_dram_parameter("out_rt", [D_MODEL, C], fp32, isOutput=True)

    # routed token chunks: multiples of 128, at most 512 (one PSUM bank each);
    # mm2 keeps all chunks of a group live in PSUM, so group chunks by 3
    chunks = []
    t0 = 0
    while t0 < C:
        w = min(TC, C - t0)
        chunks.append((t0, w))
        t0 += w
    chunk_groups = [chunks[i:i + 3] for i in range(0, len(chunks), 3)]
    # psum accumulator tags pc0/pc1 are also used (at width 512) by the
    # shared-expert mm2, so the first two chunks must be full-width
    assert C >= 1024, "expert capacity below mean load is impossible"

    with tile.TileContext(nc) as tc:
        with (
            tc.tile_pool(name="const", bufs=1) as cpool,
            tc.tile_pool(name="w1s", bufs=2) as w1pool,
            tc.tile_pool(name="w2s", bufs=2) as w2pool,
            tc.tile_pool(name="xs", bufs=3) as xpool,
            tc.tile_pool(name="shp", bufs=1) as shpool,
            tc.tile_pool(name="outp", bufs=4) as opool,
            tc.tile_pool(name="ps", bufs=2, space="PSUM") as pspool,
        ):
            # Per-k-tile DMAs throughout: one big strided DMA fans out across
            # many HW-DGE queues, and the first consuming matmul then needs
            # more sync-wait slots than walrus allows. Per-k transfers keep
            # each consumer waiting on a single queue semaphore.
            # DMA issue itself costs ~600ns of engine time per dma_start
            # (16 descriptor packets regardless of size), so transfers are
            # batched into few large calls and split between the two HW-DGE
            # issue engines (Sync for inputs, Scalar for weights + outputs).
            # Interleave the first token chunk with sw1 so the first matmul's
            # inputs land as early as possible.
            sw1_sb = cpool.tile([P, 8, HS], bf16, tag="sw1")
            xb_first = xpool.tile([P, 8, TC], bf16, tag="xb")
            for h in range(2):
                nc.sync.dma_start(sw1_sb[:, ts(h, 4), :], sw1t[:, ts(h, 4), :])
                nc.sync.dma_start(xb_first[:, ts(h, 4), :],
                                  xbf[:, ts(h, 4), ts(0, TC)])
            sw2_sb = cpool.tile([P, 4, D_MODEL], bf16, tag="sw2")
            nc.scalar.dma_start(sw2_sb[:], sw2[:])

            # ---- shared expert (hidden slice) over all T tokens ----
            with nc.named_scope("shared"):
                for c in range(T // TC):
                    if c == 0:
                        xb = xb_first
                    else:
                        xb = xpool.tile([P, 8, TC], bf16, tag="xb")
                        for h in range(2):
                            nc.sync.dma_start(xb[:, ts(h, 4), :],
                                              xbf[:, ts(h, 4), ts(c, TC)])

                    sshT = shpool.tile([P, HS // P, TC], bf16, tag="sshT")
                    for kt in range(HS // P):
                        ph = pspool.tile([P, TC], fp32, tag="ph")
                        for k in range(8):
                            nc.tensor.matmul(ph[:], sw1_sb[:, k, ts(kt, P)],
                                             xb[:, k, :],
                                             start=(k == 0), stop=(k == 7))
                        nc.scalar.activation(sshT[:, kt, :], ph[:], AF.Silu)

                    for mt in range(TC // P):
                        tt = c * (TC // P) + mt
                        psh0 = pspool.tile([P, 512], fp32, tag="pc0")
                        psh1 = pspool.tile([P, 512], fp32, tag="pc1")
                        pshs = (psh0, psh1)
                        for k in range(HS // P):
                            for nh in range(D_MODEL // 512):
                                nc.tensor.matmul(pshs[nh][:],
                                                 sshT[:, k, ts(mt, P)],
                                                 sw2_sb[:, k, ts(nh, 512)],
                                                 start=(k == 0),
                                                 stop=(k == HS // P - 1))
                        ysb = opool.tile([P, D_MODEL], bf16, tag="ysb_sh")
                        for nh in range(D_MODEL // 512):
                            nc.scalar.activation(ysb[:, ts(nh, 512)],
                                                 pshs[nh][:], AF.Copy)
                        nc.scalar.dma_start(out_sh[ds(tt * P, P), :], ysb[:])

            # gathered tokens (needed from routed mm1 onward)
            xg_sb = cpool.tile([P, 8, C], bf16, tag="xg")
            for h in range(2):
                nc.sync.dma_start(xg_sb[:, ts(h, 4), :], xg[:, ts(h, 4), :])

            # ---- routed mm1 + silu: sh.T[H, C]; w1 streamed exactly once,
            # each w1 stationary tile reused across all token chunks ----
            with nc.named_scope("mm1"):
                shT = shpool.tile([P, HIDDEN // P, C], bf16, tag="shT")
                for hg in range(HIDDEN // 512):
                    w1tile = w1pool.tile([P, 8, 512], bf16, tag="w1")
                    for h in range(2):
                        nc.scalar.dma_start(w1tile[:, ts(h, 4), :],
                                            w1t[:, ts(h, 4), ds(hg * 512, 512)])
                    for ht4 in range(4):
                        ht = hg * 4 + ht4
                        for group in chunk_groups:
                            phs = []
                            for ci, (t0, w) in enumerate(group):
                                phc = pspool.tile([P, min(w, TC)], fp32,
                                                  tag=f"pc{ci}")
                                phs.append(phc)
                            for k in range(8):
                                for ci, (t0, w) in enumerate(group):
                                    nc.tensor.matmul(
                                        phs[ci][:, :w],
                                        w1tile[:, k, ts(ht4, P)],
                                        xg_sb[:, k, ds(t0, w)],
                                        start=(k == 0), stop=(k == 7))
                            for ci, (t0, w) in enumerate(group):
                                nc.scalar.activation(shT[:, ht, ds(t0, w)],
                                                     phs[ci][:, :w], AF.Silu)

            # ---- routed mm2: yT[D, C] with w2 stationary, reused across
            # token chunks; w2 streamed exactly once ----
            with nc.named_scope("mm2"):
                for gi, group in enumerate(chunk_groups):
                    for nh in range(8):
                        w2sl = w2pool.tile([P, 32, P], bf16, tag="w2sl")
                        nc.scalar.dma_start(w2sl[:], w2r[:, nh, :])
                        pts = []
                        for ci, (t0, w) in enumerate(group):
                            pyt = pspool.tile([P, min(w, TC)], fp32,
                                              tag=f"pc{ci}")
                            pts.append(pyt)
                        for k in range(HIDDEN // P):
                            for ci, (t0, w) in enumerate(group):
                                nc.tensor.matmul(
                                    pts[ci][:, :w], w2sl[:, k, :],
                                    shT[:, k, ds(t0, w)],
                                    start=(k == 0),
                                    stop=(k == HIDDEN // P - 1))
                        for ci, (t0, w) in enumerate(group):
                            ysb = opool.tile([P, 512], fp32, tag="ysb")
                            nc.scalar.activation(ysb[:, :w], pts[ci][:, :w],
                                                 AF.Copy)
                            nc.sync.dma_start(
                                out_rt[ds(nh * P, P), ds(t0, w)],
                                ysb[:, :w])
    nc.compile()
    return nc


def _strip(a, dtype):
    # [K, F] -> [128, K//128, F] partition-major layout
    k, f = a.shape
    return np.ascontiguousarray(
        a.reshape(k // P, P, f).transpose(1, 0, 2)).astype(dtype)


def _route(x, gate_w):
    """Host-side top-2 routing, exactly matching jax.lax.top_k + softmax."""
    z = x @ gate_w.T                              # [T, E] fp32
    n = z.shape[0]
    rows = np.arange(n)
    i1 = np.argmax(z, axis=1)
    zm = z.copy()
    zm[rows, i1] = -np.inf
    i2 = np.argmax(zm, axis=1)
    v1 = z[rows, i1]
    v2 = z[rows, i2]
    e2 = np.exp((v2 - v1).astype(np.float32))
    g1 = (1.0 / (1.0 + e2)).astype(np.float32)
    g2 = (e2 / (1.0 + e2)).astype(np.float32)
    return i1, i2, g1, g2


def kernel(x, shared_w1, shared_w2, experts_w1, experts_w2, gate_w):
    global LAST_EXEC_NS, LAST_RESULT
    x = np.asarray(x, dtype=np.float32).reshape(T, D_MODEL)
    shared_w1 = np.asarray(shared_w1, dtype=np.float32)
    shared_w2 = np.asarray(shared_w2, dtype=np.float32)
    experts_w1 = np.asarray(experts_w1, dtype=np.float32)
    experts_w2 = np.asarray(experts_w2, dtype=np.float32)
    gate_w = np.asarray(gate_w, dtype=np.float32)

    xT = np.ascontiguousarray(x.T)                      # [D, T]
    xbf_prep = _strip(xT, BF16)                         # [128, 8, T]

    i1, i2, g1, g2 = _route(x, gate_w)
    idx_list, gval_list = [], []
    for c in range(N_CORES):
        idx = np.concatenate([np.nonzero(i1 == c)[0], np.nonzero(i2 == c)[0]])
        gv = np.concatenate([g1[i1 == c], g2[i2 == c]]).astype(np.float32)
        idx_list.append(idx)
        gval_list.append(gv)
    max_load = max(len(i) for i in idx_list)
    C = max(P, ((max_load + P - 1) // P) * P)

    in_maps = []
    for c in range(N_CORES):
        idx = idx_list[c]
        xg_full = np.zeros((C, D_MODEL), dtype=np.float32)
        xg_full[:len(idx)] = x[idx]
        xg_prep = _strip(np.ascontiguousarray(xg_full.T), BF16)  # [128, 8, C]

        w1t_prep = _strip(np.ascontiguousarray(experts_w1[c].T), BF16)
        w2t_prep = _strip(np.ascontiguousarray(experts_w2[c].T), BF16)
        # [128, 32k, 1024d] -> [128, 8nh, 32k, 128d] -> flatten last two
        w2r_prep = np.ascontiguousarray(
            w2t_prep.reshape(P, 32, 8, P).transpose(0, 2, 1, 3)
        ).reshape(P, 8, 32 * P)
        sw1t_prep = _strip(
            np.ascontiguousarray(shared_w1[c * HS:(c + 1) * HS, :].T), BF16)
        sw2_prep = _strip(
            np.ascontiguousarray(shared_w2[:, c * HS:(c + 1) * HS].T), BF16)
        in_maps.append({
            "xbf": xbf_prep, "xg": xg_prep,
            "w1t": w1t_prep, "w2r": w2r_prep,
            "sw1t": sw1t_prep, "sw2": sw2_prep,
        })

    nc = _build_nc(C)
    res = run_bass_kernel_spmd(nc, in_maps, list(range(N_CORES)))
    LAST_EXEC_NS = res.exec_time_ns
    LAST_RESULT = res

    total = np.zeros((T, D_MODEL), dtype=np.float32)
    for c in range(N_CORES):
        total += res.results[c]["out_sh"].astype(np.float32)
    for c in range(N_CORES):
        idx = idx_list[c]
        if len(idx):
            yt = res.results[c]["out_rt"][:, :len(idx)]        # [D, len]
            total[idx] += yt.T * gval_list[c][:, None]
    return total.reshape(2, 2048, D_MODEL).astype(np.float32)


# revision 22
# speedup vs baseline: 1.0339x; 1.0339x over previous
"""MoE feed-forward block (shared expert + top-2-of-8 routed experts) on 8
Trainium2 NeuronCores.

Sharding: expert-parallel with host-side routing (the dispatch/gather is part
of the sharding step). The host computes the top-2 gate in fp32, gathers each
expert's selected tokens (capacity-padded to a common C), and core c runs
ONLY expert c's matmuls on its ~C gathered tokens — instead of the dense
all-token compute — plus a 1/8 hidden-dim slice of the shared expert over all
tokens. Host combine: sum the 8 shared partials, scatter-add the gate-scaled
routed outputs by token index (gate scaling on host keeps the device mm2 free
to emit a transposed [d, token] layout).

Matmuls run in bf16 (fp32 accumulation in PSUM). Per-core tensor work is
~13.4 GMAC; every weight/activation byte is streamed from HBM exactly once.

Scheduling notes:
 - The shared-expert chunks are interleaved between routed-expert weight
   groups so HBM demand stays flat (~100 GB/s) instead of spiking above the
   sustainable rate during a monolithic shared phase.
 - DMA issue costs ~600ns of engine time per dma_start; input loads issue
   from Sync, output stores from Scalar (right after the ACT producing them,
   so they never head-of-line-block the silu path).
 - Per-k-slice input DMAs (128KB each) spread transfers across HW queues;
   single big calls serialize on one queue at ~100 GB/s.

Device layout:
  shared mm1: ssh.T[HS,TC] = sw1T[D,HS].T @ x.T[D,TC]    (lhsT=sw1T resident)
  shared mm2: y[TC,D]      = ssh.T[HS,TC].T @ sw2T[HS,D] (lhsT=ssh.T)
  routed mm1: sh.T[H,C]    = w1T[D,H].T @ xg.T[D,C]      (w1 streamed once)
  routed mm2: yT[D,C]      = w2T[H,D].T @ sh.T[H,C]      (lhsT=w2 stationary,
                                         reused across token chunks; output
                                         transposed so tokens are the free dim)
"""

import ml_dtypes
import numpy as np

import concourse.bass as bass
import concourse.mybir as mybir
import concourse.tile as tile
from concourse import bacc
from concourse.bass import ds, ts
from concourse.bass_utils import run_bass_kernel_spmd

BF16 = ml_dtypes.bfloat16

D_MODEL = 1024
HIDDEN = 4096
N_EXP = 8
N_CORES = 8
TOP_K = 2
T = 4096                      # 2 * 2048 tokens
HS = HIDDEN // N_CORES        # shared-expert hidden slice per core
TC = 512                      # token chunk
P = 128

LAST_EXEC_NS = None
LAST_RESULT = None


def _build_nc(C):
    fp32 = mybir.dt.float32
    bf16 = mybir.dt.bfloat16
    AF = mybir.ActivationFunctionType

    nc = bacc.Bacc()
    xbf = nc.declare_dram_parameter("xbf", [P, 8, T], bf16, isOutput=False)
    xg = nc.declare_dram_parameter("xg", [P, 8, C], bf16, isOutput=False)
    w1t = nc.declare_dram_parameter("w1t", [P, 8, HIDDEN], bf16, isOutput=False)
    # w2 reordered on host: [P, nh=8, k=32, 128] so each nh slice is one
    # contiguous 1MB DMA
    w2r = nc.declare_dram_parameter("w2r", [P, 8, 32 * P], bf16, isOutput=False)
    sw1t = nc.declare_dram_parameter("sw1t", [P, 8, HS], bf16, isOutput=False)
    sw2 = nc.declare_dram_parameter("sw2", [P, 4, D_MODEL], bf16, isOutput=False)
    out_sh = nc.declare_dram_parameter("out_sh", [T, D_MODEL], bf16, isOutput=True)
    out_rt = nc.declare_dram_parameter("out_rt", [D_MODEL, C], fp32, isOutput=True)

    # routed token chunks: multiples of 128, at most 512 (one PSUM bank each);
    # mm2 keeps all chunks of a group live in PSUM, so group chunks by 3
    chunks = []
    t0 = 0
    while t0 < C:
        w = min(TC, C - t0)
        chunks.append((t0, w))
        t0 += w
    chunk_groups = [chunks[i:i + 3] for i in range(0, len(chunks), 3)]
    # psum accumulator tags pc0/pc1 are also used (at width 512) by the
    # shared-expert mm2, so the first two chunks must be full-width
    assert C >= 1024, "expert capacity below mean load is impossible"

    with tile.TileContext(nc) as tc:
        with (
            tc.tile_pool(name="const", bufs=1) as cpool,
            tc.tile_pool(name="w1s", bufs=2) as w1pool,
            tc.tile_pool(name="w2s", bufs=2) as w2pool,
            tc.tile_pool(name="xs", bufs=3) as xpool,
            tc.tile_pool(name="shp", bufs=1) as shpool,
            tc.tile_pool(name="outp", bufs=6) as opool,
            tc.tile_pool(name="ps", bufs=2, space="PSUM") as pspool,
        ):
            # startup: batched 2-call loads (issue latency dominates here)
            sw1_sb = cpool.tile([P, 8, HS], bf16, tag="sw1")
            xb_first = xpool.tile([P, 8, TC], bf16, tag="xb")
            for h in range(2):
                nc.sync.dma_start(sw1_sb[:, ts(h, 4), :], sw1t[:, ts(h, 4), :])
                nc.sync.dma_start(xb_first[:, ts(h, 4), :],
                                  xbf[:, ts(h, 4), ts(0, TC)])
            sw2_sb = cpool.tile([P, 4, D_MODEL], bf16, tag="sw2")
            for k in range(4):
                nc.sync.dma_start(sw2_sb[:, k, :], sw2[:, k, :])
            xg_sb = cpool.tile([P, 8, C], bf16, tag="xg")
            for k in range(8):
                nc.sync.dma_start(xg_sb[:, k, :], xg[:, k, :])

            def shared_chunk(c):
                """One 512-token chunk of the shared expert (mm1+silu+mm2)."""
                if c == 0:
                    xb = xb_first
                else:
                    xb = xpool.tile([P, 8, TC], bf16, tag="xb")
                    for k in range(8):
                        nc.sync.dma_start(xb[:, k, :], xbf[:, k, ts(c, TC)])

                sshT = shpool.tile([P, HS // P, TC], bf16, tag="sshT")
                for kt in range(HS // P):
                    ph = pspool.tile([P, TC], fp32, tag="ph")
                    for k in range(8):
                        nc.tensor.matmul(ph[:], sw1_sb[:, k, ts(kt, P)],
                                         xb[:, k, :],
                                         start=(k == 0), stop=(k == 7))
                    nc.scalar.activation(sshT[:, kt, :], ph[:], AF.Silu)

                for mt in range(TC // P):
                    tt = c * (TC // P) + mt
                    psh0 = pspool.tile([P, 512], fp32, tag="pc0")
                    psh1 = pspool.tile([P, 512], fp32, tag="pc1")
                    pshs = (psh0, psh1)
                    for k in range(HS // P):
                        for nh in range(D_MODEL // 512):
                            nc.tensor.matmul(pshs[nh][:],
                                             sshT[:, k, ts(mt, P)],
                                             sw2_sb[:, k, ts(nh, 512)],
                                             start=(k == 0),
                                             stop=(k == HS // P - 1))
                    ysb = opool.tile([P, D_MODEL], bf16, tag="ysb_sh")
                    for nh in range(D_MODEL // 512):
                        nc.scalar.activation(ysb[:, ts(nh, 512)],
                                             pshs[nh][:], AF.Copy)
                    nc.scalar.dma_start(out_sh[ds(tt * P, P), :], ysb[:])

            # first shared chunk leads (its inputs land first)
            shared_chunk(0)

            # ---- routed mm1 + silu interleaved with shared chunks 1..4 ----
            shT = shpool.tile([P, HIDDEN // P, C], bf16, tag="shT")
            for hg in range(HIDDEN // 512):
                w1tile = w1pool.tile([P, 8, 512], bf16, tag="w1")
                for k in range(8):
                    nc.sync.dma_start(w1tile[:, k, :],
                                      w1t[:, k, ds(hg * 512, 512)])
                if hg % 2 == 0 and hg // 2 + 1 <= 4:
                    shared_chunk(hg // 2 + 1)
                for ht4 in range(4):
                    ht = hg * 4 + ht4
                    for group in chunk_groups:
                        phs = []
                        for ci, (t0, w) in enumerate(group):
                            phc = pspool.tile([P, min(w, TC)], fp32,
                                              tag=f"pc{ci}")
                            phs.append(phc)
                        for k in range(8):
                            for ci, (t0, w) in enumerate(group):
                                nc.tensor.matmul(
                                    phs[ci][:, :w],
                                    w1tile[:, k, ts(ht4, P)],
                                    xg_sb[:, k, ds(t0, w)],
                                    start=(k == 0), stop=(k == 7))
                        for ci, (t0, w) in enumerate(group):
                            nc.scalar.activation(shT[:, ht, ds(t0, w)],
                                                 phs[ci][:, :w], AF.Silu)

            # ---- routed mm2 interleaved with shared chunks 5..7; w2
            # stationary reused across token chunks, streamed once ----
            for gi, group in enumerate(chunk_groups):
                for nh in range(8):
                    w2sl = w2pool.tile([P, 32, P], bf16, tag="w2sl")
                    nc.sync.dma_start(w2sl[:], w2r[:, nh, :])
                    if gi == 0 and nh % 2 == 0 and nh // 2 + 5 <= 7:
                        shared_chunk(nh // 2 + 5)
                    pts = []
                    for ci, (t0, w) in enumerate(group):
                        pyt = pspool.tile([P, min(w, TC)], fp32,
                                          tag=f"pc{ci}")
                        pts.append(pyt)
                    for k in range(HIDDEN // P):
                        for ci, (t0, w) in enumerate(group):
                            nc.tensor.matmul(
                                pts[ci][:, :w], w2sl[:, k, :],
                                shT[:, k, ds(t0, w)],
                                start=(k == 0),
                                stop=(k == HIDDEN // P - 1))
                    for ci, (t0, w) in enumerate(group):
                        ysb = opool.tile([P, 512], fp32, tag="ysb")
                        nc.scalar.activation(ysb[:, :w], pts[ci][:, :w],
                                             AF.Copy)
                        nc.scalar.dma_start(
                            out_rt[ds(nh * P, P), ds(t0, w)],
                            ysb[:, :w])
    nc.compile()
    return nc


def _strip(a, dtype):
    # [K, F] -> [128, K//128, F] partition-major layout
    k, f = a.shape
    return np.ascontiguousarray(
        a.reshape(k // P, P, f).transpose(1, 0, 2)).astype(dtype)


def _route(x, gate_w):
    """Host-side top-2 routing, exactly matching jax.lax.top_k + softmax."""
    z = x @ gate_w.T                              # [T, E] fp32
    n = z.shape[0]
    rows = np.arange(n)
    i1 = np.argmax(z, axis=1)
    zm = z.copy()
    zm[rows, i1] = -np.inf
    i2 = np.argmax(zm, axis=1)
    v1 = z[rows, i1]
    v2 = z[rows, i2]
    e2 = np.exp((v2 - v1).astype(np.float32))
    g1 = (1.0 / (1.0 + e2)).astype(np.float32)
    g2 = (e2 / (1.0 + e2)).astype(np.float32)
    return i1, i2, g1, g2


def kernel(x, shared_w1, shared_w2, experts_w1, experts_w2, gate_w):
    global LAST_EXEC_NS, LAST_RESULT
    x = np.asarray(x, dtype=np.float32).reshape(T, D_MODEL)
    shared_w1 = np.asarray(shared_w1, dtype=np.float32)
    shared_w2 = np.asarray(shared_w2, dtype=np.float32)
    experts_w1 = np.asarray(experts_w1, dtype=np.float32)
    experts_w2 = np.asarray(experts_w2, dtype=np.float32)
    gate_w = np.asarray(gate_w, dtype=np.float32)

    xT = np.ascontiguousarray(x.T)                      # [D, T]
    xbf_prep = _strip(xT, BF16)                         # [128, 8, T]

    i1, i2, g1, g2 = _route(x, gate_w)
    idx_list, gval_list = [], []
    for c in range(N_CORES):
        idx = np.concatenate([np.nonzero(i1 == c)[0], np.nonzero(i2 == c)[0]])
        gv = np.concatenate([g1[i1 == c], g2[i2 == c]]).astype(np.float32)
        idx_list.append(idx)
        gval_list.append(gv)
    max_load = max(len(i) for i in idx_list)
    C = max(2 * TC, ((max_load + P - 1) // P) * P)

    in_maps = []
    for c in range(N_CORES):
        idx = idx_list[c]
        xg_full = np.zeros((C, D_MODEL), dtype=np.float32)
        xg_full[:len(idx)] = x[idx]
        xg_prep = _strip(np.ascontiguousarray(xg_full.T), BF16)  # [128, 8, C]

        w1t_prep = _strip(np.ascontiguousarray(experts_w1[c].T), BF16)
        w2t_prep = _strip(np.ascontiguousarray(experts_w2[c].T), BF16)
        # [128, 32k, 1024d] -> [128, 8nh, 32k, 128d] -> flatten last two
        w2r_prep = np.ascontiguousarray(
            w2t_prep.reshape(P, 32, 8, P).transpose(0, 2, 1, 3)
        ).reshape(P, 8, 32 * P)
        sw1t_prep = _strip(
            np.ascontiguousarray(shared_w1[c * HS:(c + 1) * HS, :].T), BF16)
        sw2_prep = _strip(
            np.ascontiguousarray(shared_w2[:, c * HS:(c + 1) * HS].T), BF16)
        in_maps.append({
            "xbf": xbf_prep, "xg": xg_prep,
            "w1t": w1t_prep, "w2r": w2r_prep,
            "sw1t": sw1t_prep, "sw2": sw2_prep,
        })

    nc = _build_nc(C)
    res = run_bass_kernel_spmd(nc, in_maps, list(range(N_CORES)))
    LAST_EXEC_NS = res.exec_time_ns
    LAST_RESULT = res

    total = np.zeros((T, D_MODEL), dtype=np.float32)
    for c in range(N_CORES):
        total += res.results[c]["out_sh"].astype(np.float32)
    for c in range(N_CORES):
        idx = idx_list[c]
        if len(idx):
            yt = res.results[c]["out_rt"][:, :len(idx)]        # [D, len]
            total[idx] += yt.T * gval_list[c][:, None]
    return total.reshape(2, 2048, D_MODEL).astype(np.float32)


# revision 26
# speedup vs baseline: 1.0365x; 1.0025x over previous
"""MoE feed-forward block (shared expert + top-2-of-8 routed experts) on 8
Trainium2 NeuronCores.

Sharding: expert-parallel with host-side routing (the dispatch/gather is part
of the sharding step). The host computes the top-2 gate in fp32, gathers each
expert's selected tokens (capacity-padded to a common C), and core c runs
ONLY expert c's matmuls on its ~C gathered tokens — instead of the dense
all-token compute — plus a 1/8 hidden-dim slice of the shared expert over all
tokens. Host combine: sum the 8 shared partials, scatter-add the gate-scaled
routed outputs by token index (gate scaling on host keeps the device mm2 free
to emit a transposed [d, token] layout).

Matmuls run in bf16 (fp32 accumulation in PSUM). Per-core tensor work is
~13.4 GMAC; every weight/activation byte is streamed from HBM exactly once.

Scheduling notes:
 - The shared-expert chunks are interleaved between routed-expert weight
   groups so HBM demand stays flat (~100 GB/s) instead of spiking above the
   sustainable rate during a monolithic shared phase.
 - DMA issue costs ~600ns of engine time per dma_start; input loads issue
   from Sync, output stores from Scalar (right after the ACT producing them,
   so they never head-of-line-block the silu path).
 - Per-k-slice input DMAs (128KB each) spread transfers across HW queues;
   single big calls serialize on one queue at ~100 GB/s.

Device layout:
  shared mm1: ssh.T[HS,TC] = sw1T[D,HS].T @ x.T[D,TC]    (lhsT=sw1T resident)
  shared mm2: y[TC,D]      = ssh.T[HS,TC].T @ sw2T[HS,D] (lhsT=ssh.T)
  routed mm1: sh.T[H,C]    = w1T[D,H].T @ xg.T[D,C]      (w1 streamed once)
  routed mm2: yT[D,C]      = w2T[H,D].T @ sh.T[H,C]      (lhsT=w2 stationary,
                                         reused across token chunks; output
                                         transposed so tokens are the free dim)
"""

import ml_dtypes
import numpy as np

import concourse.bass as bass
import concourse.mybir as mybir
import concourse.tile as tile
from concourse import bacc
from concourse.bass import ds, ts
from concourse.bass_utils import run_bass_kernel_spmd

BF16 = ml_dtypes.bfloat16

D_MODEL = 1024
HIDDEN = 4096
N_EXP = 8
N_CORES = 8
TOP_K = 2
T = 4096                      # 2 * 2048 tokens
HS = HIDDEN // N_CORES        # shared-expert hidden slice per core
TC = 512                      # token chunk
P = 128

LAST_EXEC_NS = None
LAST_RESULT = None


def _build_nc(C):
    fp32 = mybir.dt.float32
    bf16 = mybir.dt.bfloat16
    AF = mybir.ActivationFunctionType

    nc = bacc.Bacc()
    xbf = nc.declare_dram_parameter("xbf", [P, 8, T], bf16, isOutput=False)
    xg = nc.declare_dram_parameter("xg", [P, 8, C], bf16, isOutput=False)
    w1t = nc.declare_dram_parameter("w1t", [P, 8, HIDDEN], bf16, isOutput=False)
    # w2 reordered on host: [P, nh=8, k=32, 128] so each nh slice is one
    # contiguous 1MB DMA
    w2r = nc.declare_dram_parameter("w2r", [P, 8, 32 * P], bf16, isOutput=False)
    sw1t = nc.declare_dram_parameter("sw1t", [P, 8, HS], bf16, isOutput=False)
    sw2 = nc.declare_dram_parameter("sw2", [P, 4, D_MODEL], bf16, isOutput=False)
    out_sh = nc.declare_dram_parameter("out_sh", [T, D_MODEL], bf16, isOutput=True)
    out_rt = nc.declare_dram_parameter("out_rt", [D_MODEL, C], fp32, isOutput=True)

    # routed token chunks: multiples of 128, at most 512 (one PSUM bank each);
    # mm2 keeps all chunks of a group live in PSUM, so group chunks by 3
    chunks = []
    t0 = 0
    while t0 < C:
        w = min(TC, C - t0)
        chunks.append((t0, w))
        t0 += w
    chunk_groups = [chunks[i:i + 3] for i in range(0, len(chunks), 3)]
    # psum accumulator tags pc0/pc1 are also used (at width 512) by the
    # shared-expert mm2, so the first two chunks must be full-width
    assert C >= 1024, "expert capacity below mean load is impossible"

    with tile.TileContext(nc) as tc:
        with (
            tc.tile_pool(name="const", bufs=1) as cpool,
            tc.tile_pool(name="w1s", bufs=2) as w1pool,
            tc.tile_pool(name="w2s", bufs=2) as w2pool,
            tc.tile_pool(name="xs", bufs=3) as xpool,
            tc.tile_pool(name="shp", bufs=1) as shpool,
            tc.tile_pool(name="outp", bufs=6) as opool,
            tc.tile_pool(name="ps", bufs=2, space="PSUM") as pspool,
        ):
            # startup: batched 2-call loads (issue latency dominates here)
            sw1_sb = cpool.tile([P, 8, HS], bf16, tag="sw1")
            xb_first = xpool.tile([P, 8, TC], bf16, tag="xb")
            for h in range(2):
                nc.sync.dma_start(sw1_sb[:, ts(h, 4), :], sw1t[:, ts(h, 4), :])
                nc.sync.dma_start(xb_first[:, ts(h, 4), :],
                                  xbf[:, ts(h, 4), ts(0, TC)])
            sw2_sb = cpool.tile([P, 4, D_MODEL], bf16, tag="sw2")
            for h in range(2):
                nc.sync.dma_start(sw2_sb[:, ts(h, 2), :], sw2[:, ts(h, 2), :])
            # xg issues from Scalar so it doesn't delay the Sync queue's
            # xb/w1 prefetches during the startup window
            xg_sb = cpool.tile([P, 8, C], bf16, tag="xg")
            for k in range(8):
                nc.scalar.dma_start(xg_sb[:, k, :], xg[:, k, :])

            def shared_chunk(c):
                """One 512-token chunk of the shared expert (mm1+silu+mm2)."""
                if c == 0:
                    xb = xb_first
                else:
                    xb = xpool.tile([P, 8, TC], bf16, tag="xb")
                    for k in range(8):
                        nc.sync.dma_start(xb[:, k, :], xbf[:, k, ts(c, TC)])

                sshT = shpool.tile([P, HS // P, TC], bf16, tag="sshT")
                for kt in range(HS // P):
                    ph = pspool.tile([P, TC], fp32, tag="ph")
                    for k in range(8):
                        nc.tensor.matmul(ph[:], sw1_sb[:, k, ts(kt, P)],
                                         xb[:, k, :],
                                         start=(k == 0), stop=(k == 7))
                    nc.scalar.activation(sshT[:, kt, :], ph[:], AF.Silu)

                for mt in range(TC // P):
                    tt = c * (TC // P) + mt
                    psh0 = pspool.tile([P, 512], fp32, tag="pc0")
                    psh1 = pspool.tile([P, 512], fp32, tag="pc1")
                    pshs = (psh0, psh1)
                    for k in range(HS // P):
                        for nh in range(D_MODEL // 512):
                            nc.tensor.matmul(pshs[nh][:],
                                             sshT[:, k, ts(mt, P)],
                                             sw2_sb[:, k, ts(nh, 512)],
                                             start=(k == 0),
                                             stop=(k == HS // P - 1))
                    ysb = opool.tile([P, D_MODEL], bf16, tag="ysb_sh")
                    for nh in range(D_MODEL // 512):
                        nc.vector.tensor_scalar_mul(ysb[:, ts(nh, 512)],
                                                    pshs[nh][:], 1.0)
                    nc.scalar.dma_start(out_sh[ds(tt * P, P), :], ysb[:])

            # first shared chunk leads (its inputs land first)
            shared_chunk(0)

            # ---- routed mm1 + silu interleaved with shared chunks 1..4 ----
            shT = shpool.tile([P, HIDDEN // P, C], bf16, tag="shT")
            for hg in range(HIDDEN // 512):
                # the shared chunk runs on the PE before this hg's mm1, so
                # its xb must also issue before this hg's w1
                if hg % 2 == 0 and hg // 2 + 1 <= 4:
                    shared_chunk(hg // 2 + 1)
                w1tile = w1pool.tile([P, 8, 512], bf16, tag="w1")
                for k in range(8):
                    nc.sync.dma_start(w1tile[:, k, :],
                                      w1t[:, k, ds(hg * 512, 512)])
                for ht4 in range(4):
                    ht = hg * 4 + ht4
                    for group in chunk_groups:
                        phs = []
                        for ci, (t0, w) in enumerate(group):
                            phc = pspool.tile([P, min(w, TC)], fp32,
                                              tag=f"pc{ci}")
                            phs.append(phc)
                        for k in range(8):
                            for ci, (t0, w) in enumerate(group):
                                nc.tensor.matmul(
                                    phs[ci][:, :w],
                                    w1tile[:, k, ts(ht4, P)],
                                    xg_sb[:, k, ds(t0, w)],
                                    start=(k == 0), stop=(k == 7))
                        for ci, (t0, w) in enumerate(group):
                            nc.scalar.activation(shT[:, ht, ds(t0, w)],
                                                 phs[ci][:, :w], AF.Silu)

            # ---- routed mm2 interleaved with shared chunks 5..7; w2
            # stationary reused across token chunks, streamed once ----
            for gi, group in enumerate(chunk_groups):
                for nh in range(8):
                    w2sl = w2pool.tile([P, 32, P], bf16, tag="w2sl")
                    nc.sync.dma_start(w2sl[:], w2r[:, nh, :])
                    if gi == 0 and nh % 2 == 0 and nh // 2 + 5 <= 7:
                        shared_chunk(nh // 2 + 5)
                    pts = []
                    for ci, (t0, w) in enumerate(group):
                        pyt = pspool.tile([P, min(w, TC)], fp32,
                                          tag=f"pc{ci}")
                        pts.append(pyt)
                    for k in range(HIDDEN // P):
                        for ci, (t0, w) in enumerate(group):
                            nc.tensor.matmul(
                                pts[ci][:, :w], w2sl[:, k, :],
                                shT[:, k, ds(t0, w)],
                                start=(k == 0),
                                stop=(k == HIDDEN // P - 1))
                    for ci, (t0, w) in enumerate(group):
                        ysb = opool.tile([P, 512], fp32, tag="ysb")
                        nc.vector.tensor_scalar_mul(ysb[:, :w],
                                                    pts[ci][:, :w], 1.0)
                        nc.scalar.dma_start(
                            out_rt[ds(nh * P, P), ds(t0, w)],
                            ysb[:, :w])
    nc.compile()
    return nc


def _strip(a, dtype):
    # [K, F] -> [128, K//128, F] partition-major layout
    k, f = a.shape
    return np.ascontiguousarray(
        a.reshape(k // P, P, f).transpose(1, 0, 2)).astype(dtype)


def _route(x, gate_w):
    """Host-side top-2 routing, exactly matching jax.lax.top_k + softmax."""
    z = x @ gate_w.T                              # [T, E] fp32
    n = z.shape[0]
    rows = np.arange(n)
    i1 = np.argmax(z, axis=1)
    zm = z.copy()
    zm[rows, i1] = -np.inf
    i2 = np.argmax(zm, axis=1)
    v1 = z[rows, i1]
    v2 = z[rows, i2]
    e2 = np.exp((v2 - v1).astype(np.float32))
    g1 = (1.0 / (1.0 + e2)).astype(np.float32)
    g2 = (e2 / (1.0 + e2)).astype(np.float32)
    return i1, i2, g1, g2


def kernel(x, shared_w1, shared_w2, experts_w1, experts_w2, gate_w):
    global LAST_EXEC_NS, LAST_RESULT
    x = np.asarray(x, dtype=np.float32).reshape(T, D_MODEL)
    shared_w1 = np.asarray(shared_w1, dtype=np.float32)
    shared_w2 = np.asarray(shared_w2, dtype=np.float32)
    experts_w1 = np.asarray(experts_w1, dtype=np.float32)
    experts_w2 = np.asarray(experts_w2, dtype=np.float32)
    gate_w = np.asarray(gate_w, dtype=np.float32)

    xT = np.ascontiguousarray(x.T)                      # [D, T]
    xbf_prep = _strip(xT, BF16)                         # [128, 8, T]

    i1, i2, g1, g2 = _route(x, gate_w)
    idx_list, gval_list = [], []
    for c in range(N_CORES):
        idx = np.concatenate([np.nonzero(i1 == c)[0], np.nonzero(i2 == c)[0]])
        gv = np.concatenate([g1[i1 == c], g2[i2 == c]]).astype(np.float32)
        idx_list.append(idx)
        gval_list.append(gv)
    max_load = max(len(i) for i in idx_list)
    C = max(2 * TC, ((max_load + P - 1) // P) * P)

    in_maps = []
    for c in range(N_CORES):
        idx = idx_list[c]
        xg_full = np.zeros((C, D_MODEL), dtype=np.float32)
        xg_full[:len(idx)] = x[idx]
        xg_prep = _strip(np.ascontiguousarray(xg_full.T), BF16)  # [128, 8, C]

        w1t_prep = _strip(np.ascontiguousarray(experts_w1[c].T), BF16)
        w2t_prep = _strip(np.ascontiguousarray(experts_w2[c].T), BF16)
        # [128, 32k, 1024d] -> [128, 8nh, 32k, 128d] -> flatten last two
        w2r_prep = np.ascontiguousarray(
            w2t_prep.reshape(P, 32, 8, P).transpose(0, 2, 1, 3)
        ).reshape(P, 8, 32 * P)
        sw1t_prep = _strip(
            np.ascontiguousarray(shared_w1[c * HS:(c + 1) * HS, :].T), BF16)
        sw2_prep = _strip(
            np.ascontiguousarray(shared_w2[:, c * HS:(c + 1) * HS].T), BF16)
        in_maps.append({
            "xbf": xbf_prep, "xg": xg_prep,
            "w1t": w1t_prep, "w2r": w2r_prep,
            "sw1t": sw1t_prep, "sw2": sw2_prep,
        })

    nc = _build_nc(C)
    res = run_bass_kernel_spmd(nc, in_maps, list(range(N_CORES)))
    LAST_EXEC_NS = res.exec_time_ns
    LAST_RESULT = res

    total = np.zeros((T, D_MODEL), dtype=np.float32)
    for c in range(N_CORES):
        total += res.results[c]["out_sh"].astype(np.float32)
    for c in range(N_CORES):
        idx = idx_list[c]
        if len(idx):
            yt = res.results[c]["out_rt"][:, :len(idx)]        # [D, len]
            total[idx] += yt.T * gval_list[c][:, None]
    return total.reshape(2, 2048, D_MODEL).astype(np.float32)
